# revision 2
# baseline (speedup 1.0000x reference)
"""Trainium2 Bass kernel for nn_CustomConvLayer (3x3-tap conv).

out[b,o,h,w] = sum_c sum_k x_pad[b,c,h+dh_k,w+dw_k] * weights[o,c,k]
x: [16,64,128,128] f32, weights: [128,64,9] f32 -> out: [16,128,128,128] f32

Strategy (8 NeuronCores, data-parallel over batch, 2 images/core):
- Host pre-pads each image plane to 130x130 and lays the two images of a
  core on SBUF partitions 0-63 (img0) and 64-127 (img1).
- TensorE runs two concurrent 64x128 row-tiled matmul streams (tile
  positions (0,0) and (64,0)), one per image, contracting C=64 per tap.
  9 taps accumulate into one PSUM bank per (image, 4-row group); tap
  shifts are pure AP offsets into the padded plane - no im2col, no data
  duplication.
- float32r matmuls: full TensorE rate (1 row/cycle for N=512) at ~1.5e-4
  relative error vs fp32 reference.
- x is DMA-loaded in 10 row-chunks so compute starts after the first
  chunk; PSUM is evacuated by VectorE into 16-row staging tiles and
  stored with 1MiB DMAs on the scalar-engine HWDGE ring (loads use the
  sync-engine ring).
"""
import numpy as np

C, O, H, Wd, Wp, KT, NIMG, R, G = 64, 128, 128, 128, 130, 9, 2, 4, 32
NCORES = 8
TAPS = [(dh, dw) for dh in range(3) for dw in range(3)]

_CACHE = {}


def _build(rep=1):
    from concourse import bacc
    import concourse.mybir as mybir
    from concourse.tile import TileContext

    f32 = mybir.dt.float32
    f32r = mybir.dt.float32r

    nc = bacc.Bacc()
    xp = nc.declare_dram_parameter("xp", [NIMG * C, Wp * Wp], f32r, isOutput=False)
    wp = nc.declare_dram_parameter("wp", [128, KT * O], f32r, isOutput=False)
    out = nc.declare_dram_parameter("out", [NIMG, O, H, Wd], f32, isOutput=True)

    NCHUNK = 10
    CROWS = Wp // NCHUNK  # 13 rows per load chunk

    with TileContext(nc) as tc:
        with tc.tile_pool(name="xpool", bufs=1) as xpool, \
             tc.tile_pool(name="wpool", bufs=1) as wpool, \
             tc.tile_pool(name="spool", bufs=3) as spool, \
             tc.tile_pool(name="ps", bufs=8, space="PSUM") as pspool:
            xt = xpool.tile([128, Wp * Wp], f32r)
            wt = wpool.tile([128, KT * O], f32r)
            nc.sync.dma_start(out=wt[:], in_=wp[:])
            xv = xt[:].rearrange("p (r w) -> p r w", w=Wp)
            wv = wt[:].rearrange("p (k o) -> p k o", o=O)

            def body():
                for ck in range(NCHUNK):
                    o0 = ck * CROWS * Wp
                    o1 = o0 + CROWS * Wp
                    nc.sync.dma_start(out=xt[:, o0:o1], in_=xp[:, o0:o1])
                st = [None, None]
                for g in range(G):
                    h0 = g * R
                    q = g % 4
                    if q == 0:
                        st = [spool.tile([128, 4 * R * Wd], f32, tag="st",
                                         name=f"st{g}_{i}") for i in range(NIMG)]
                    pst = [pspool.tile([128, R * Wd], f32, tag="ps",
                                       name=f"ps{g}_{i}") for i in range(NIMG)]
                    for t, (dh, dw) in enumerate(TAPS):
                        for img in range(NIMG):
                            b = img * 64
                            rhs = xv[b:b + 64, h0 + dh:h0 + dh + R, dw:dw + Wd]
                            lhsT = wv[b:b + 64, t, :]
                            nc.tensor.matmul(pst[img][:], lhsT, rhs,
                                             start=(t == 0), stop=(t == KT - 1))
                    for img in range(NIMG):
                        nc.vector.tensor_copy(st[img][:, q * R * Wd:(q + 1) * R * Wd],
                                              pst[img][:])
                        if q == 3:
                            hs = (g // 4) * 16
                            nc.scalar.dma_start(out=out[img, :, hs:hs + 16, :],
                                                in_=st[img][:])

            if rep == 1:
                body()
            else:
                with tc.For_i(0, rep, 1, hint_engines=(mybir.EngineType.PE,)):
                    body()
    nc.compile()
    return nc


def _get_nc(rep=1):
    if rep not in _CACHE:
        _CACHE[rep] = _build(rep)
    return _CACHE[rep]


def _prep_maps(x, weights):
    x = np.ascontiguousarray(x, dtype=np.float32)
    w = np.ascontiguousarray(weights, dtype=np.float32)
    w_t = np.ascontiguousarray(w.transpose(1, 2, 0)).reshape(C, KT * O)
    wp = np.concatenate([w_t, w_t], axis=0)
    B = x.shape[0]
    xpad = np.zeros((B, C, Wp, Wp), np.float32)
    xpad[:, :, 1:1 + H, 1:1 + Wd] = x
    maps = []
    for c in range(NCORES):
        xs = xpad[c * NIMG:(c + 1) * NIMG]
        maps.append({"xp": np.ascontiguousarray(xs).reshape(NIMG * C, Wp * Wp),
                     "wp": wp})
    return maps


def kernel(x, weights):
    from concourse.bass_utils import run_bass_kernel_spmd

    nc = _get_nc()
    maps = _prep_maps(x, weights)
    res = run_bass_kernel_spmd(nc, maps, list(range(NCORES)))
    return np.concatenate([res.results[c]["out"] for c in range(NCORES)], axis=0)


# revision 3
# speedup vs baseline: 1.0112x; 1.0112x over previous
"""Trainium2 Bass kernel for nn_CustomConvLayer (3x3-tap conv).

out[b,o,h,w] = sum_c sum_k x_pad[b,c,h+dh_k,w+dw_k] * weights[o,c,k]
x: [16,64,128,128] f32, weights: [128,64,9] f32 -> out: [16,128,128,128] f32

Strategy (8 NeuronCores, data-parallel over batch, 2 images/core):
- Host pre-pads each image plane to 130x130 and lays the two images of a
  core on SBUF partitions 0-63 (img0) and 64-127 (img1).
- TensorE runs two concurrent 64x128 row-tiled matmul streams (tile
  positions (0,0) and (64,0)), one per image, contracting C=64 per tap.
  9 taps accumulate into one PSUM bank per (image, 4-row group); tap
  shifts are pure AP offsets into the padded plane - no im2col, no data
  duplication.
- float32r matmuls: full TensorE rate (1 row/cycle for N=512) at ~1.5e-4
  relative error vs fp32 reference.
- x is DMA-loaded in 10 row-chunks so compute starts after the first
  chunk (Tile subtile deps let matmuls chase the loads); PSUM is
  evacuated by VectorE into 16-row staging tiles and stored with 1MiB
  DMAs on the scalar-engine HWDGE ring (loads use the sync-engine ring).

Measured on TRN2 (8 cores concurrent, per-core): ~84 us steady-state for
the full body (PE-bound; DMA ~70 us hidden underneath), rel err ~1.5e-4
vs the fp32 reference. Plain fp32 matmuls would be 3.4x slower (280 us);
bf16 would save ~20% wall but cost 2.5e-3 error.
"""
import numpy as np

C, O, H, Wd, Wp, KT, NIMG, R, G = 64, 128, 128, 128, 130, 9, 2, 4, 32
NCORES = 8
TAPS = [(dh, dw) for dh in range(3) for dw in range(3)]

_CACHE = {}


def _build(rep=1):
    from concourse import bacc
    import concourse.mybir as mybir
    from concourse.tile import TileContext

    f32 = mybir.dt.float32
    f32r = mybir.dt.float32r

    nc = bacc.Bacc()
    xp = nc.declare_dram_parameter("xp", [NIMG * C, Wp * Wp], f32r, isOutput=False)
    wp = nc.declare_dram_parameter("wp", [128, KT * O], f32r, isOutput=False)
    out = nc.declare_dram_parameter("out", [NIMG, O, H, Wd], f32, isOutput=True)

    NCHUNK = 10
    CROWS = Wp // NCHUNK  # 13 rows per load chunk

    with TileContext(nc) as tc:
        with tc.tile_pool(name="xpool", bufs=1) as xpool, \
             tc.tile_pool(name="wpool", bufs=1) as wpool, \
             tc.tile_pool(name="spool", bufs=3) as spool, \
             tc.tile_pool(name="ps", bufs=8, space="PSUM") as pspool:
            xt = xpool.tile([128, Wp * Wp], f32r)
            wt = wpool.tile([128, KT * O], f32r)
            nc.sync.dma_start(out=wt[:], in_=wp[:])
            xv = xt[:].rearrange("p (r w) -> p r w", w=Wp)
            wv = wt[:].rearrange("p (k o) -> p k o", o=O)

            def body():
                for ck in range(NCHUNK):
                    o0 = ck * CROWS * Wp
                    o1 = o0 + CROWS * Wp
                    nc.sync.dma_start(out=xt[:, o0:o1], in_=xp[:, o0:o1])
                st = [None, None]
                for g in range(G):
                    h0 = g * R
                    q = g % 4
                    if q == 0:
                        st = [spool.tile([128, 4 * R * Wd], f32, tag="st",
                                         name=f"st{g}_{i}") for i in range(NIMG)]
                    pst = [pspool.tile([128, R * Wd], f32, tag="ps",
                                       name=f"ps{g}_{i}") for i in range(NIMG)]
                    for t, (dh, dw) in enumerate(TAPS):
                        for img in range(NIMG):
                            b = img * 64
                            rhs = xv[b:b + 64, h0 + dh:h0 + dh + R, dw:dw + Wd]
                            lhsT = wv[b:b + 64, t, :]
                            nc.tensor.matmul(pst[img][:], lhsT, rhs,
                                             start=(t == 0), stop=(t == KT - 1))
                    for img in range(NIMG):
                        nc.vector.tensor_copy(st[img][:, q * R * Wd:(q + 1) * R * Wd],
                                              pst[img][:])
                        if q == 3:
                            hs = (g // 4) * 16
                            nc.scalar.dma_start(out=out[img, :, hs:hs + 16, :],
                                                in_=st[img][:])

            if rep == 1:
                body()
            else:
                with tc.For_i(0, rep, 1, hint_engines=(mybir.EngineType.PE,)):
                    body()
    nc.compile()
    return nc


def _get_nc(rep=1):
    if rep not in _CACHE:
        _CACHE[rep] = _build(rep)
    return _CACHE[rep]


def _prep_maps(x, weights):
    x = np.ascontiguousarray(x, dtype=np.float32)
    w = np.ascontiguousarray(weights, dtype=np.float32)
    w_t = np.ascontiguousarray(w.transpose(1, 2, 0)).reshape(C, KT * O)
    wp = np.concatenate([w_t, w_t], axis=0)
    B = x.shape[0]
    xpad = np.zeros((B, C, Wp, Wp), np.float32)
    xpad[:, :, 1:1 + H, 1:1 + Wd] = x
    maps = []
    for c in range(NCORES):
        xs = xpad[c * NIMG:(c + 1) * NIMG]
        maps.append({"xp": np.ascontiguousarray(xs).reshape(NIMG * C, Wp * Wp),
                     "wp": wp})
    return maps


def kernel(x, weights):
    from concourse.bass_utils import run_bass_kernel_spmd

    nc = _get_nc()
    maps = _prep_maps(x, weights)
    res = run_bass_kernel_spmd(nc, maps, list(range(NCORES)))
    return np.concatenate([res.results[c]["out"] for c in range(NCORES)], axis=0)


# revision 8
# speedup vs baseline: 1.0123x; 1.0012x over previous
"""Trainium2 Bass kernel for nn_CustomConvLayer (3x3-tap conv).

out[b,o,h,w] = sum_c sum_k x_pad[b,c,h+dh_k,w+dw_k] * weights[o,c,k]
x: [16,64,128,128] f32, weights: [128,64,9] f32 -> out: [16,128,128,128] f32

Strategy (8 NeuronCores, data-parallel over batch, 2 images/core):
- Host pre-pads each image plane to 130x130 and lays the two images of a
  core on SBUF partitions 0-63 (img0) and 64-127 (img1).
- TensorE runs two concurrent 64x128 row-tiled matmul streams (tile
  positions (0,0) and (64,0)), one per image, contracting C=64 per tap.
  9 taps accumulate into one PSUM bank per (image, 4-row group); tap
  shifts are pure AP offsets into the padded plane - no im2col, no data
  duplication.
- float32r matmuls: full TensorE rate (1 row/cycle for N=512) at ~1.5e-4
  relative error vs fp32 reference.
- x is DMA-loaded in row-chunks (small first chunk) so compute starts
  ~1.5us in (Tile subtile deps let matmuls chase the loads); dummy
  warm-up matmuls run during the load window so the PE HAM clock gate
  is already 8/8 when real work starts; PSUM is
  evacuated by VectorE into 16-row staging tiles and stored with 1MiB
  DMAs on the scalar-engine HWDGE ring (loads use the sync-engine ring).

Measured on TRN2 (8 cores concurrent, per-core): ~84 us steady-state for
the full body (PE-bound; DMA ~70 us hidden underneath), rel err ~1.5e-4
vs the fp32 reference. Plain fp32 matmuls would be 3.4x slower (280 us);
bf16 would save ~20% wall but cost 2.5e-3 error.
"""
import numpy as np

C, O, H, Wd, Wp, KT, NIMG, R, G = 64, 128, 128, 128, 130, 9, 2, 4, 32
NCORES = 8
TAPS = [(dh, dw) for dh in range(3) for dw in range(3)]

_CACHE = {}


def _build(rep=1):
    from concourse import bacc
    import concourse.mybir as mybir
    from concourse.tile import TileContext

    f32 = mybir.dt.float32
    f32r = mybir.dt.float32r

    nc = bacc.Bacc()
    xp = nc.declare_dram_parameter("xp", [NIMG * C, Wp * Wp], f32r, isOutput=False)
    wp = nc.declare_dram_parameter("wp", [128, KT * O], f32r, isOutput=False)
    out = nc.declare_dram_parameter("out", [NIMG, O, H, Wd], f32, isOutput=True)

    # x row-chunks: small first chunk so group-0 matmuls start ~1.5us in;
    # the rest sized for DMA efficiency. Rows must sum to Wp=130.
    CHUNK_ROWS = [6] + [14] * 8 + [12]

    with TileContext(nc) as tc:
        with tc.tile_pool(name="xpool", bufs=1) as xpool, \
             tc.tile_pool(name="wpool", bufs=1) as wpool, \
             tc.tile_pool(name="spool", bufs=3) as spool, \
             tc.tile_pool(name="ps", bufs=8, space="PSUM") as pspool:
            xt = xpool.tile([128, Wp * Wp], f32r)
            wt = wpool.tile([128, KT * O], f32r)
            # weights on the gpsimd (SWDGE) ring so they land in parallel
            # with the sync-ring x chunk loads
            nc.gpsimd.dma_start(out=wt[:], in_=wp[:])
            xv = xt[:].rearrange("p (r w) -> p r w", w=Wp)
            wv = wt[:].rearrange("p (k o) -> p k o", o=O)

            def body(it=0):
                row = 0
                for ck, nrows in enumerate(CHUNK_ROWS):
                    o0 = row * Wp
                    o1 = o0 + nrows * Wp
                    nc.sync.dma_start(out=xt[:, o0:o1], in_=xp[:, o0:o1])
                    row += nrows
                # HAM warm-up: dummy matmuls on the weight tile while the
                # first x chunks are still in flight, so the PE clock gate is
                # at 8/8 when real work starts. Results land in group 0's
                # PSUM bank and are discarded by the start=True reset below.
                warm = pspool.tile([128, R * Wd], f32, tag="ps", name=f"warm{it}")
                for wi in range(8):
                    nc.tensor.matmul(warm[:], wv[0:64, 0, :], wt[0:64, 0:512],
                                     start=True, stop=True)
                st = [None, None]
                for g in range(G):
                    h0 = g * R
                    q = g % 4
                    if q == 0:
                        st = [spool.tile([128, 4 * R * Wd], f32, tag="st",
                                         name=f"st{g}_{i}") for i in range(NIMG)]
                    pst = [pspool.tile([128, R * Wd], f32, tag="ps",
                                       name=f"ps{g}_{i}") for i in range(NIMG)]
                    for t, (dh, dw) in enumerate(TAPS):
                        for img in range(NIMG):
                            b = img * 64
                            rhs = xv[b:b + 64, h0 + dh:h0 + dh + R, dw:dw + Wd]
                            lhsT = wv[b:b + 64, t, :]
                            nc.tensor.matmul(pst[img][:], lhsT, rhs,
                                             start=(t == 0), stop=(t == KT - 1))
                    for img in range(NIMG):
                        nc.vector.tensor_copy(st[img][:, q * R * Wd:(q + 1) * R * Wd],
                                              pst[img][:])
                        if q == 3:
                            hs = (g // 4) * 16
                            nc.scalar.dma_start(out=out[img, :, hs:hs + 16, :],
                                                in_=st[img][:])

            if rep == 1:
                body()
            else:
                with tc.For_i(0, rep, 1, hint_engines=(mybir.EngineType.PE,)):
                    body()
    nc.compile()
    return nc


def _get_nc(rep=1):
    if rep not in _CACHE:
        _CACHE[rep] = _build(rep)
    return _CACHE[rep]


def _prep_maps(x, weights):
    x = np.ascontiguousarray(x, dtype=np.float32)
    w = np.ascontiguousarray(weights, dtype=np.float32)
    w_t = np.ascontiguousarray(w.transpose(1, 2, 0)).reshape(C, KT * O)
    wp = np.concatenate([w_t, w_t], axis=0)
    B = x.shape[0]
    xpad = np.zeros((B, C, Wp, Wp), np.float32)
    xpad[:, :, 1:1 + H, 1:1 + Wd] = x
    maps = []
    for c in range(NCORES):
        xs = xpad[c * NIMG:(c + 1) * NIMG]
        maps.append({"xp": np.ascontiguousarray(xs).reshape(NIMG * C, Wp * Wp),
                     "wp": wp})
    return maps


def kernel(x, weights):
    from concourse.bass_utils import run_bass_kernel_spmd

    nc = _get_nc()
    maps = _prep_maps(x, weights)
    res = run_bass_kernel_spmd(nc, maps, list(range(NCORES)))
    return np.concatenate([res.results[c]["out"] for c in range(NCORES)], axis=0)


# revision 9
# speedup vs baseline: 1.1049x; 1.0915x over previous
"""Trainium2 Bass kernel for nn_CustomConvLayer (3x3-tap conv).

out[b,o,h,w] = sum_c sum_k x_pad[b,c,h+dh_k,w+dw_k] * weights[o,c,k]
x: [16,64,128,128] f32, weights: [128,64,9] f32 -> out: [16,128,128,128] f32

Strategy (8 NeuronCores, data-parallel over batch, 2 images/core):
- Host pre-pads each image plane to 130x130 and lays the two images of a
  core on SBUF partitions 0-63 (img0) and 64-127 (img1).
- TensorE runs two concurrent 64x128 row-tiled matmul streams (tile
  positions (0,0) and (64,0)), one per image, contracting C=64 per tap.
  9 taps accumulate into one PSUM bank per (image, 4-row group); tap
  shifts are pure AP offsets into the padded plane - no im2col, no data
  duplication.
- float32r matmuls: full TensorE rate (1 row/cycle for N=512) at ~1.5e-4
  relative error vs fp32 reference.
- x is DMA-loaded in row-chunks (small first chunk) so compute starts
  ~1.5us in (Tile subtile deps let matmuls chase the loads); dummy
  warm-up matmuls run during the load window so the PE HAM clock gate
  is already 8/8 when real work starts; PSUM is
  evacuated by VectorE into 16-row staging tiles and stored with 1MiB
  DMAs on the scalar-engine HWDGE ring (loads use the sync-engine ring).

Measured on TRN2 (8 cores concurrent, per-core): ~84 us steady-state for
the full body (PE-bound; DMA ~70 us hidden underneath), rel err ~1.5e-4
vs the fp32 reference. Plain fp32 matmuls would be 3.4x slower (280 us);
bf16 would save ~20% wall but cost 2.5e-3 error.
"""
import numpy as np

C, O, H, Wd, Wp, KT, NIMG, R, G = 64, 128, 128, 128, 130, 9, 2, 4, 32
NCORES = 8
TAPS = [(dh, dw) for dh in range(3) for dw in range(3)]

_CACHE = {}


def _build(rep=1):
    from concourse import bacc
    import concourse.mybir as mybir
    from concourse.tile import TileContext

    f32 = mybir.dt.float32
    f32r = mybir.dt.float32r

    nc = bacc.Bacc()
    xp = nc.declare_dram_parameter("xp", [NIMG * C, Wp * Wp], f32r, isOutput=False)
    wp = nc.declare_dram_parameter("wp", [128, KT * O], f32r, isOutput=False)
    out = nc.declare_dram_parameter("out", [NIMG, O, H, Wd], f32, isOutput=True)

    # x row-chunks: small first chunk so group-0 matmuls start ~1.5us in;
    # the rest sized for DMA efficiency. Rows must sum to Wp=130.
    CHUNK_ROWS = [6] + [14] * 8 + [12]

    with TileContext(nc) as tc:
        with tc.tile_pool(name="xpool", bufs=1) as xpool, \
             tc.tile_pool(name="wpool", bufs=1) as wpool, \
             tc.tile_pool(name="spool", bufs=3) as spool, \
             tc.tile_pool(name="ps", bufs=8, space="PSUM") as pspool:
            xt = xpool.tile([128, Wp * Wp], f32r)
            wt = wpool.tile([128, KT * O], f32r)
            # weights on the gpsimd (SWDGE) ring so they land in parallel
            # with the sync-ring x chunk loads
            nc.gpsimd.dma_start(out=wt[:], in_=wp[:])
            xv = xt[:].rearrange("p (r w) -> p r w", w=Wp)
            wv = wt[:].rearrange("p (k o) -> p k o", o=O)

            def body(it=0):
                row = 0
                for ck, nrows in enumerate(CHUNK_ROWS):
                    o0 = row * Wp
                    o1 = o0 + nrows * Wp
                    nc.sync.dma_start(out=xt[:, o0:o1], in_=xp[:, o0:o1])
                    row += nrows
                # HAM warm-up: dummy matmuls on the weight tile while the
                # first x chunks are still in flight, so the PE clock gate is
                # at 8/8 when real work starts. Results land in group 0's
                # PSUM bank and are discarded by the start=True reset below.
                warm = pspool.tile([128, R * Wd], f32, tag="ps", name=f"warm{it}")
                for wi in range(8):
                    nc.tensor.matmul(warm[:], wv[0:64, 0, :], wt[0:64, 0:512],
                                     start=True, stop=True)
                st = [None, None]
                for g in range(G):
                    h0 = g * R
                    q = g % 4
                    if q == 0:
                        st = [spool.tile([128, 4 * R * Wd], f32, tag="st",
                                         name=f"st{g}_{i}") for i in range(NIMG)]
                    pst = [pspool.tile([128, R * Wd], f32, tag="ps",
                                       name=f"ps{g}_{i}") for i in range(NIMG)]
                    for t, (dh, dw) in enumerate(TAPS):
                        for img in range(NIMG):
                            b = img * 64
                            rhs = xv[b:b + 64, h0 + dh:h0 + dh + R, dw:dw + Wd]
                            lhsT = wv[b:b + 64, t, :]
                            nc.tensor.matmul(pst[img][:], lhsT, rhs,
                                             start=(t == 0), stop=(t == KT - 1))
                    last_block = g >= G - 4
                    for img in range(NIMG):
                        nc.vector.tensor_copy(st[img][:, q * R * Wd:(q + 1) * R * Wd],
                                              pst[img][:])
                        if last_block:
                            # final block: store per-group (256KB) so the
                            # post-compute DMA tail is ~1.2us, not ~3.5us
                            nc.scalar.dma_start(out=out[img, :, h0:h0 + R, :],
                                                in_=st[img][:, q * R * Wd:(q + 1) * R * Wd])
                        elif q == 3:
                            hs = (g // 4) * 16
                            nc.scalar.dma_start(out=out[img, :, hs:hs + 16, :],
                                                in_=st[img][:])

            if rep == 1:
                body()
            else:
                with tc.For_i(0, rep, 1, hint_engines=(mybir.EngineType.PE,)):
                    body()
    nc.compile()
    return nc


def _get_nc(rep=1):
    if rep not in _CACHE:
        _CACHE[rep] = _build(rep)
    return _CACHE[rep]


def _prep_maps(x, weights):
    x = np.ascontiguousarray(x, dtype=np.float32)
    w = np.ascontiguousarray(weights, dtype=np.float32)
    w_t = np.ascontiguousarray(w.transpose(1, 2, 0)).reshape(C, KT * O)
    wp = np.concatenate([w_t, w_t], axis=0)
    B = x.shape[0]
    xpad = np.zeros((B, C, Wp, Wp), np.float32)
    xpad[:, :, 1:1 + H, 1:1 + Wd] = x
    maps = []
    for c in range(NCORES):
        xs = xpad[c * NIMG:(c + 1) * NIMG]
        maps.append({"xp": np.ascontiguousarray(xs).reshape(NIMG * C, Wp * Wp),
                     "wp": wp})
    return maps


def kernel(x, weights):
    from concourse.bass_utils import run_bass_kernel_spmd

    nc = _get_nc()
    maps = _prep_maps(x, weights)
    res = run_bass_kernel_spmd(nc, maps, list(range(NCORES)))
    return np.concatenate([res.results[c]["out"] for c in range(NCORES)], axis=0)
